# revision 21
# baseline (speedup 1.0000x reference)
"""Supervised-contrastive loss on 8 Trainium2 NeuronCores.

Math (reference):
    z = x / max(||x||, 1e-8)                  row-normalize
    sim = (z @ z.T) / TEMP                    [N, N]
    per-anchor: pos-mean over same-class (excl. self) and logsumexp over
    j != i, then per-class mean, then mean over classes.

Distribution: rows of z (anchors) are sharded 1024/core across 8 cores;
z is replicated.  Each core computes its [1024, 8192] slab of raw cosine
similarities (fp8-e4m3 inputs via DoubleRow matmuls, fp32 PSUM
accumulate) and reduces it on the fly to two tiny outputs:
    esp[i, g] = sum_{j in group g} exp(10 * sim[i, j])   (incl. diagonal)
    tm[i, c]  = sum_{j in class c} sim[i, j]             (incl. diagonal)
The exp row-sums ride on the ScalarE activation via accum_out (no extra
reduction pass).  The class-segment sums are folded into a small GEMM:
tm = A @ W.T where W[c] = sum of z rows of class c (precomputed on
host), so no masking is needed on device.  The diagonal sim[i,i] =
||z8[i]||^2 is reconstructed exactly on host and subtracted there.  The
final logsumexp/segment-mean arithmetic over 8192 anchors is negligible
host work.

Layout: all fp8 operands are host-packed for DoubleRow so that feature
d = kk*256 + i*128 + p lands on partition p, plane i of contraction tile
kk, giving 2-6KB-contiguous per-partition DMA descriptors.

Hardware pitfalls baked into this structure (each verified by a device
crash or a multi-us regression): DMAs only from nc.sync (scalar-HWDGE
and gpsimd-SWDGE both misbehave), one matmul accumulation group per
PSUM bank, and one EXP per psum tile (slice-wise EXP serializes the PE).
"""

import numpy as np
import ml_dtypes

N = 8192          # anchors
D = 768           # feature dim
NOP = 64          # number of classes
CORES = 8
SLAB = N // CORES  # 1024 anchors per core
KT8 = D // 256     # 3 double-row contraction tiles
MT = SLAB // 128   # 8 anchor chunks of 128 (PSUM partition dim)
JW = 512           # matmul free width (one PSUM bank)
GW = 2048          # j-group width (one wide PSUM tile / DMA chunk)
NG = N // GW       # 4 groups
GJ = GW // JW      # 4 matmul slices per group
TEMP_INV = 10.0
EPS = 1e-8

FP8 = ml_dtypes.float8_e4m3

_CACHE = {}
LAST_RESULT = None  # BassKernelResults of the most recent run (for profiling)


def _build_nc():
    from concourse import bacc
    import concourse.mybir as mybir
    import concourse.tile as tile

    f8 = mybir.dt.float8e4
    f32 = mybir.dt.float32
    Exp = mybir.ActivationFunctionType.Exp
    DR = mybir.MatmulPerfMode.DoubleRow

    nc = bacc.Bacc(
        "TRN2", target_bir_lowering=False, debug=False, enable_asserts=False
    )
    z8 = nc.dram_tensor("z8", [128, KT8, NG, 2, GW], f8, kind="ExternalInput").ap()
    a8 = nc.dram_tensor("a8", [128, KT8, 2, SLAB], f8, kind="ExternalInput").ap()
    w8 = nc.dram_tensor("w8", [128, KT8, 2, NOP], f8, kind="ExternalInput").ap()
    pout = nc.dram_tensor("pout", [128, MT * NG + 2], f32, kind="ExternalOutput").ap()
    tm = nc.dram_tensor("tm", [128, MT, NOP], f32, kind="ExternalOutput").ap()

    with tile.TileContext(nc) as tc:
        with (
            tc.tile_pool(name="zin", bufs=KT8 * NG) as zin,
            tc.tile_pool(name="singles", bufs=1) as singles,
        ):
            # ---- input DMAs (small/early operands first) ----
            w8_sb = singles.tile([128, KT8, 2, NOP], f8)
            nc.sync.dma_start(out=w8_sb, in_=w8)
            a8_sb = singles.tile([128, KT8, 2, SLAB], f8)

            def dma_a8_half(half):
                if half > 0:
                    return
                nc.sync.dma_start(
                    out=a8_sb.rearrange("p a b c -> p (a b c)"),
                    in_=a8.rearrange("p a b c -> p (a b c)"),
                )

            z8_sb = {}

            def dma_z8_group(g):
                for kk in range(KT8):
                    z8_t = zin.tile([128, 2, GW], f8, name="z8_t", tag="z8_t")
                    nc.sync.dma_start(out=z8_t, in_=z8[:, kk, g, :, :])
                    z8_sb[(g, kk)] = z8_t

            dma_a8_half(0)
            dma_a8_half(1)
            for g in range(NG):
                dma_z8_group(g)

            pacc = singles.tile([128, MT * NG + 2], f32)
            # the last (g, m) iteration writes the two tail slots instead
            nc.vector.memset(pacc[:, MT * NG - 1:MT * NG], 0.0)
            tm_sb = singles.tile([128, MT, NOP], f32)

            ps_pool = tc.alloc_tile_pool(name="ps", bufs=2, space="PSUM")

            # ---- class-segment sums: tm[:, m, c] = A_m @ W.T ----
            for m in range(MT):
                pst = ps_pool.tile([128, NOP], f32, name="ps_t", tag="ps_t")
                for kk in range(KT8):
                    nc.tensor.matmul(
                        pst,
                        a8_sb[:, kk, :, m * 128:(m + 1) * 128],
                        w8_sb[:, kk, :, :],
                        start=(kk == 0),
                        stop=(kk == KT8 - 1),
                        perf_mode=DR,
                    )
                nc.vector.tensor_copy(tm_sb[:, m, :], pst)
            nc.sync.dma_start(out=tm, in_=tm_sb)

            # ---- main similarity slab (fp8 DoubleRow) + fused exp sums ----
            for g in range(NG):
                for m in range(MT):
                    last = (g == NG - 1) and (m == MT - 1)
                    if not last:
                        ps_t = ps_pool.tile([128, GW], f32, name="ps_t", tag="ps_t")
                        for kk in range(KT8):
                            lhsT = a8_sb[:, kk, :, m * 128:(m + 1) * 128]
                            for jj in range(GJ):
                                nc.tensor.matmul(
                                    ps_t[:, jj * JW:(jj + 1) * JW],
                                    lhsT,
                                    z8_sb[(g, kk)][:, :, jj * JW:(jj + 1) * JW],
                                    start=(kk == 0),
                                    stop=(kk == KT8 - 1),
                                    perf_mode=DR,
                                )
                        nc.scalar.activation(
                            out=ps_t,
                            in_=ps_t,
                            func=Exp,
                            scale=TEMP_INV,
                            accum_out=pacc[:, m * NG + g:m * NG + g + 1],
                        )
                    else:
                        # final iteration: two half-width tiles from the same
                        # slots, so the first EXP starts before the last MMs
                        # finish (separate tiles -> no intra-tile serialization)
                        for h in range(2):
                            ps_h = ps_pool.tile(
                                [128, GW // 2], f32, name="ps_t", tag="ps_t"
                            )
                            for kk in range(KT8):
                                lhsT = a8_sb[:, kk, :, m * 128:(m + 1) * 128]
                                for jj in range(2):
                                    j = h * 2 + jj
                                    nc.tensor.matmul(
                                        ps_h[:, jj * JW:(jj + 1) * JW],
                                        lhsT,
                                        z8_sb[(g, kk)][:, :, j * JW:(j + 1) * JW],
                                        start=(kk == 0),
                                        stop=(kk == KT8 - 1),
                                        perf_mode=DR,
                                    )
                            nc.scalar.activation(
                                out=ps_h,
                                in_=ps_h,
                                func=Exp,
                                scale=TEMP_INV,
                                accum_out=pacc[:, MT * NG + h:MT * NG + h + 1],
                            )
            ps_pool.release()

            nc.sync.dma_start(out=pout, in_=pacc)

    nc.compile()
    return nc


def _get_nc():
    if "nc" not in _CACHE:
        _CACHE["nc"] = _build_nc()
    return _CACHE["nc"]


def _pack_dr(mat_t):
    """[D, cols] -> [128, KT8, 2, cols] with d = kk*256 + i*128 + p."""
    d, cols = mat_t.shape
    return np.ascontiguousarray(
        mat_t.reshape(KT8, 2, 128, cols).transpose(2, 0, 1, 3)
    )


def kernel(x, op_ids, n_op):
    global LAST_RESULT
    from concourse.bass_utils import run_bass_kernel_spmd

    x = np.asarray(x, dtype=np.float32).reshape(-1, D)
    op_ids = np.asarray(op_ids).reshape(-1).astype(np.int64)
    n_op_i = int(np.asarray(n_op))

    # ---- host prep: normalize, quantize, class sums, diagonal ----
    norms = np.sqrt((x.astype(np.float64) ** 2).sum(axis=1))
    norms = np.maximum(norms, EPS).astype(np.float32)
    z = x / norms[:, None]

    z8 = z.astype(FP8)
    z8f = z8.astype(np.float32)

    onehot = np.zeros((N, NOP), np.float32)
    onehot[np.arange(N), op_ids] = 1.0
    W8 = (onehot.T @ z8f).astype(FP8)               # [NOP, D] fp8

    z8_packed = _pack_dr(np.ascontiguousarray(z8.T))          # [128,3,2,N]
    # [128, KT8, NG, 2, GW]: each (g, kk) chunk contiguous per partition
    z8_chunked = np.ascontiguousarray(
        z8_packed.reshape(128, KT8, 2, NG, GW).transpose(0, 1, 3, 2, 4)
    )
    w8_packed = _pack_dr(np.ascontiguousarray(W8.T.astype(FP8)))
    ssq = (z8f.astype(np.float64) ** 2).sum(axis=1)  # = sim[i, i]

    in_maps = [
        {
            "z8": z8_chunked,
            "a8": np.ascontiguousarray(z8_packed[:, :, :, c * SLAB:(c + 1) * SLAB]),
            "w8": w8_packed,
        }
        for c in range(CORES)
    ]

    nc = _get_nc()
    res = run_bass_kernel_spmd(nc, in_maps, core_ids=list(range(CORES)))
    LAST_RESULT = res

    # ---- host post: stitch slabs, subtract diagonal, finish loss ----
    es_slabs = []
    tm_slabs = []
    for c in range(CORES):
        pout_c = res.results[c]["pout"].astype(np.float64)  # [128, MT*NG+2]
        esp_c = pout_c[:, :MT * NG].reshape(128, MT, NG)
        es_c = esp_c.sum(axis=2)
        # last (g, m) iteration wrote its two half-sums to the extra slots
        es_c[:, MT - 1] = (
            esp_c[:, MT - 1, :NG - 1].sum(axis=1) + pout_c[:, MT * NG:].sum(axis=1)
        )
        es_slabs.append(es_c.T.reshape(SLAB))
        tm_slabs.append(
            res.results[c]["tm"].transpose(1, 0, 2).reshape(SLAB, NOP)
        )
    es_full = np.concatenate(es_slabs)
    tm_full = np.concatenate(tm_slabs).astype(np.float64)

    lse = np.log(es_full - np.exp(TEMP_INV * ssq))
    pos_sum = TEMP_INV * (tm_full[np.arange(N), op_ids] - ssq)
    counts = np.bincount(op_ids, minlength=n_op_i).astype(np.float64)
    pos_cnt = counts[op_ids] - 1.0

    loss_i = np.where(pos_cnt > 0, -pos_sum / np.maximum(pos_cnt, 1.0) + lse, 0.0)
    cls_sum = np.bincount(op_ids, weights=loss_i, minlength=n_op_i)
    cls_loss = np.where(counts > 0, cls_sum / np.maximum(counts, 1.0), 0.0)
    return np.float32(cls_loss.mean())


# revision 22
# speedup vs baseline: 1.0010x; 1.0010x over previous
"""Supervised-contrastive loss on 8 Trainium2 NeuronCores.

Math (reference):
    z = x / max(||x||, 1e-8)                  row-normalize
    sim = (z @ z.T) / TEMP                    [N, N]
    per-anchor: pos-mean over same-class (excl. self) and logsumexp over
    j != i, then per-class mean, then mean over classes.

Distribution: rows of z (anchors) are sharded 1024/core across 8 cores;
z is replicated.  Each core computes its [1024, 8192] slab of raw cosine
similarities (fp8-e4m3 inputs via DoubleRow matmuls, fp32 PSUM
accumulate) and reduces it on the fly to two tiny outputs:
    esp[i, g] = sum_{j in group g} exp(10 * sim[i, j])   (incl. diagonal)
    tm[i, c]  = sum_{j in class c} sim[i, j]             (incl. diagonal)
The exp row-sums ride on the ScalarE activation via accum_out (no extra
reduction pass).  The class-segment sums are folded into a small GEMM:
tm = A @ W.T where W[c] = sum of z rows of class c (precomputed on
host), so no masking is needed on device.  The diagonal sim[i,i] =
||z8[i]||^2 is reconstructed exactly on host and subtracted there.  The
final logsumexp/segment-mean arithmetic over 8192 anchors is negligible
host work.

Layout: all fp8 operands are host-packed for DoubleRow so that feature
d = kk*256 + i*128 + p lands on partition p, plane i of contraction tile
kk, giving 2-6KB-contiguous per-partition DMA descriptors.

Hardware pitfalls baked into this structure (each verified by a device
crash or a multi-us regression): DMAs only from nc.sync (scalar-HWDGE
and gpsimd-SWDGE both misbehave), one matmul accumulation group per
PSUM bank, full-128-partition DoubleRow outputs only (M=64 out crashes),
and one EXP per psum tile (slice-wise EXP serializes the PE).
"""

import numpy as np
import ml_dtypes

N = 8192          # anchors
D = 768           # feature dim
NOP = 64          # number of classes
CORES = 8
SLAB = N // CORES  # 1024 anchors per core
KT8 = D // 256     # 3 double-row contraction tiles
MT = SLAB // 128   # 8 anchor chunks of 128 (PSUM partition dim)
JW = 512           # matmul free width (one PSUM bank)
GW = 2048          # j-group width (one wide PSUM tile / DMA chunk)
NG = N // GW       # 4 groups
GJ = GW // JW      # 4 matmul slices per group
TEMP_INV = 10.0
EPS = 1e-8

FP8 = ml_dtypes.float8_e4m3

_CACHE = {}
LAST_RESULT = None  # BassKernelResults of the most recent run (for profiling)


def _build_nc():
    from concourse import bacc
    import concourse.mybir as mybir
    import concourse.tile as tile

    f8 = mybir.dt.float8e4
    f32 = mybir.dt.float32
    Exp = mybir.ActivationFunctionType.Exp
    DR = mybir.MatmulPerfMode.DoubleRow

    nc = bacc.Bacc(
        "TRN2", target_bir_lowering=False, debug=False, enable_asserts=False
    )
    z8 = nc.dram_tensor("z8", [128, KT8, NG, 2, GW], f8, kind="ExternalInput").ap()
    a8 = nc.dram_tensor("a8", [128, KT8, 2, SLAB], f8, kind="ExternalInput").ap()
    w8 = nc.dram_tensor("w8", [128, KT8, 2, NOP], f8, kind="ExternalInput").ap()
    pout = nc.dram_tensor("pout", [128, MT * NG + 2], f32, kind="ExternalOutput").ap()
    tm = nc.dram_tensor("tm", [128, MT, NOP], f32, kind="ExternalOutput").ap()

    with tile.TileContext(nc) as tc:
        with (
            tc.tile_pool(name="zin", bufs=KT8 * NG) as zin,
            tc.tile_pool(name="singles", bufs=1) as singles,
        ):
            # ---- input DMAs (small/early operands first) ----
            w8_sb = singles.tile([128, KT8, 2, NOP], f8)
            nc.sync.dma_start(out=w8_sb, in_=w8)
            a8_sb = singles.tile([128, KT8, 2, SLAB], f8)

            def dma_a8_half(half):
                if half > 0:
                    return
                nc.sync.dma_start(
                    out=a8_sb.rearrange("p a b c -> p (a b c)"),
                    in_=a8.rearrange("p a b c -> p (a b c)"),
                )

            z8_sb = {}

            def dma_z8_group(g):
                for kk in range(KT8):
                    z8_t = zin.tile([128, 2, GW], f8, name="z8_t", tag="z8_t")
                    nc.sync.dma_start(out=z8_t, in_=z8[:, kk, g, :, :])
                    z8_sb[(g, kk)] = z8_t

            dma_a8_half(0)
            dma_a8_half(1)
            for g in range(NG):
                dma_z8_group(g)

            pacc = singles.tile([128, MT * NG + 2], f32)
            # the last (g, m) iteration writes the two tail slots instead
            nc.vector.memset(pacc[:, MT * NG - 1:MT * NG], 0.0)
            tm_sb = singles.tile([128, MT, NOP], f32)

            ps_pool = tc.alloc_tile_pool(name="ps", bufs=2, space="PSUM")

            # ---- class-segment sums: tm[:, m, c] = A_m @ W.T ----
            for m in range(MT):
                pst = ps_pool.tile([128, NOP], f32, name="ps_t", tag="ps_t")
                for kk in range(KT8):
                    nc.tensor.matmul(
                        pst,
                        a8_sb[:, kk, :, m * 128:(m + 1) * 128],
                        w8_sb[:, kk, :, :],
                        start=(kk == 0),
                        stop=(kk == KT8 - 1),
                        perf_mode=DR,
                    )
                nc.vector.tensor_copy(tm_sb[:, m, :], pst)
            nc.sync.dma_start(out=tm, in_=tm_sb)

            # ---- main similarity slab (fp8 DoubleRow) + fused exp sums ----
            for g in range(NG):
                for m in range(MT):
                    last = (g == NG - 1) and (m == MT - 1)
                    if not last:
                        ps_t = ps_pool.tile([128, GW], f32, name="ps_t", tag="ps_t")
                        for kk in range(KT8):
                            lhsT = a8_sb[:, kk, :, m * 128:(m + 1) * 128]
                            for jj in range(GJ):
                                nc.tensor.matmul(
                                    ps_t[:, jj * JW:(jj + 1) * JW],
                                    lhsT,
                                    z8_sb[(g, kk)][:, :, jj * JW:(jj + 1) * JW],
                                    start=(kk == 0),
                                    stop=(kk == KT8 - 1),
                                    perf_mode=DR,
                                )
                        nc.scalar.activation(
                            out=ps_t,
                            in_=ps_t,
                            func=Exp,
                            scale=TEMP_INV,
                            accum_out=pacc[:, m * NG + g:m * NG + g + 1],
                        )
                    else:
                        # final iteration: two half-width tiles from the same
                        # slots, so the first EXP starts before the last MMs
                        # finish (separate tiles -> no intra-tile serialization)
                        for h in range(2):
                            ps_h = ps_pool.tile(
                                [128, GW // 2], f32, name="ps_t", tag="ps_t"
                            )
                            for kk in range(KT8):
                                lhsT = a8_sb[:, kk, :, m * 128:(m + 1) * 128]
                                for jj in range(2):
                                    j = h * 2 + jj
                                    nc.tensor.matmul(
                                        ps_h[:, jj * JW:(jj + 1) * JW],
                                        lhsT,
                                        z8_sb[(g, kk)][:, :, j * JW:(j + 1) * JW],
                                        start=(kk == 0),
                                        stop=(kk == KT8 - 1),
                                        perf_mode=DR,
                                    )
                            nc.scalar.activation(
                                out=ps_h,
                                in_=ps_h,
                                func=Exp,
                                scale=TEMP_INV,
                                accum_out=pacc[:, MT * NG + h:MT * NG + h + 1],
                            )
            ps_pool.release()

            nc.sync.dma_start(out=pout, in_=pacc)

    nc.compile()
    return nc


def _get_nc():
    if "nc" not in _CACHE:
        _CACHE["nc"] = _build_nc()
    return _CACHE["nc"]


def _pack_dr(mat_t):
    """[D, cols] -> [128, KT8, 2, cols] with d = kk*256 + i*128 + p."""
    d, cols = mat_t.shape
    return np.ascontiguousarray(
        mat_t.reshape(KT8, 2, 128, cols).transpose(2, 0, 1, 3)
    )


def kernel(x, op_ids, n_op):
    global LAST_RESULT
    from concourse.bass_utils import run_bass_kernel_spmd

    x = np.asarray(x, dtype=np.float32).reshape(-1, D)
    op_ids = np.asarray(op_ids).reshape(-1).astype(np.int64)
    n_op_i = int(np.asarray(n_op))

    # ---- host prep: normalize, quantize, class sums, diagonal ----
    norms = np.sqrt((x.astype(np.float64) ** 2).sum(axis=1))
    norms = np.maximum(norms, EPS).astype(np.float32)
    z = x / norms[:, None]

    z8 = z.astype(FP8)
    z8f = z8.astype(np.float32)

    onehot = np.zeros((N, NOP), np.float32)
    onehot[np.arange(N), op_ids] = 1.0
    W8 = (onehot.T @ z8f).astype(FP8)               # [NOP, D] fp8

    z8_packed = _pack_dr(np.ascontiguousarray(z8.T))          # [128,3,2,N]
    # [128, KT8, NG, 2, GW]: each (g, kk) chunk contiguous per partition
    z8_chunked = np.ascontiguousarray(
        z8_packed.reshape(128, KT8, 2, NG, GW).transpose(0, 1, 3, 2, 4)
    )
    w8_packed = _pack_dr(np.ascontiguousarray(W8.T.astype(FP8)))
    ssq = (z8f.astype(np.float64) ** 2).sum(axis=1)  # = sim[i, i]

    in_maps = [
        {
            "z8": z8_chunked,
            "a8": np.ascontiguousarray(z8_packed[:, :, :, c * SLAB:(c + 1) * SLAB]),
            "w8": w8_packed,
        }
        for c in range(CORES)
    ]

    nc = _get_nc()
    res = run_bass_kernel_spmd(nc, in_maps, core_ids=list(range(CORES)))
    LAST_RESULT = res

    # ---- host post: stitch slabs, subtract diagonal, finish loss ----
    es_slabs = []
    tm_slabs = []
    for c in range(CORES):
        pout_c = res.results[c]["pout"].astype(np.float64)  # [128, MT*NG+2]
        esp_c = pout_c[:, :MT * NG].reshape(128, MT, NG)
        es_c = esp_c.sum(axis=2)
        # last (g, m) iteration wrote its two half-sums to the extra slots
        es_c[:, MT - 1] = (
            esp_c[:, MT - 1, :NG - 1].sum(axis=1) + pout_c[:, MT * NG:].sum(axis=1)
        )
        es_slabs.append(es_c.T.reshape(SLAB))
        tm_slabs.append(
            res.results[c]["tm"].transpose(1, 0, 2).reshape(SLAB, NOP)
        )
    es_full = np.concatenate(es_slabs)
    tm_full = np.concatenate(tm_slabs).astype(np.float64)

    lse = np.log(es_full - np.exp(TEMP_INV * ssq))
    pos_sum = TEMP_INV * (tm_full[np.arange(N), op_ids] - ssq)
    counts = np.bincount(op_ids, minlength=n_op_i).astype(np.float64)
    pos_cnt = counts[op_ids] - 1.0

    loss_i = np.where(pos_cnt > 0, -pos_sum / np.maximum(pos_cnt, 1.0) + lse, 0.0)
    cls_sum = np.bincount(op_ids, weights=loss_i, minlength=n_op_i)
    cls_loss = np.where(counts > 0, cls_sum / np.maximum(counts, 1.0), 0.0)
    return np.float32(cls_loss.mean())
